# revision 31
# baseline (speedup 1.0000x reference)
"""GCN (3-layer, PyG GCNConv-style) forward on 8 Trainium2 NeuronCores.

Strategy: data-parallel over the 64 graphs (8 graphs per core).  The
message-passing scatter-add is a dense normalized-adjacency matmul run in
fp8e4m3 with MatmulPerfMode.DoubleRow (two 128-row k-tiles per instruction,
0.5 cycles/row), which is 4x the fp32r FLOP rate for the dominant A@h
product.  Weight-side matmuls stay bf16 (weight quantization error is
systematic across nodes and does not average out; fp8 weights blow the
error budget, bf16 lands ~1.6e-3 on the logits).

Host-side prep: the feature gather from the 500k-row table and the dense
A^T build happen on the host; the device receives per-graph feature tiles
(bf16, feature-major [128, 2048]) and A^T tiles (fp8, [128 src-part,
16 src-chunk, 2048 dst] swizzle) so each graph needs exactly two large
contiguous DMAs (features on the Pool DGE queue, A^T on SP so their
fixed per-DMA delays overlap).  Per layer on device:
    h   = x @ W        (16 bf16 matmuls, 4-chunk PSUM groups, bulk-cast
                        to fp8 on DVE)
    x'  = relu(A @ h + b)  (32 fp8 DoubleRow matmuls into 4 psum strips,
                        relu+bias on ACT writing bf16)
The layer orientations alternate (feat-major <-> node-major) so no
transposes are needed anywhere.  The device emits only the per-graph
node-sum accumulators (ACT accum_out); the 64x2 logits head and the
log_softmax run on the host.
"""

import os
import sys

for _p in ("/opt/trn_rl_repo", "/root/.axon_site/_ro/trn_rl_repo"):
    if os.path.isdir(_p) and _p not in sys.path:
        sys.path.insert(0, _p)

import numpy as np
import ml_dtypes

import concourse.bass as bass
import concourse.bacc as bacc
import concourse.mybir as mybir
import concourse.tile as tile
from concourse import bass2jax

G, N, E = 64, 2048, 32768
D = H = 128
O = 2
ALL = 500_000
P = 128
N_CORES = 8
GPC = G // N_CORES          # graphs per core
NCH = N // P                # 128-row chunks per graph (16)

f32 = mybir.dt.float32
bf16 = mybir.dt.bfloat16
f8 = mybir.dt.float8e4

E4NP = ml_dtypes.float8_e4m3      # == mybir.dt.np(float8e4)
BFNP = ml_dtypes.bfloat16

DR = mybir.MatmulPerfMode.DoubleRow
RELU = mybir.ActivationFunctionType.Relu


def _build_program(n_layers: int):
    nc = bacc.Bacc("TRN2", target_bir_lowering=False, debug=False,
                   num_devices=N_CORES)

    # packed fp8 weight-compensation pairs [W8 | (W - W8)8]: wres, gw[l]s,
    # wfc.  Every W-side matmul runs fp8 DoubleRow with the input broadcast
    # (stride-0) across the two k-tiles, recovering ~bf16 weight precision
    # at 2x the bf16 matmul rate.
    NW = 2 + n_layers
    x0 = nc.dram_tensor("x0", [P, GPC * N], f8, kind="ExternalInput")
    at = nc.dram_tensor("at", [GPC * P, NCH * N], f8, kind="ExternalInput")
    wpk = nc.dram_tensor("wpk", [P, NW * 2 * H], f8, kind="ExternalInput")
    bpk = nc.dram_tensor("bpk", [P, NW], f32, kind="ExternalInput")
    macc_out = nc.dram_tensor("macc_out", [P, GPC * 4], f32,
                              kind="ExternalOutput")

    with tile.TileContext(nc) as tc:
        with tc.tile_pool(name="const", bufs=1) as const, \
             tc.tile_pool(name="apool", bufs=2) as apool, \
             tc.tile_pool(name="inpool", bufs=2) as inpool, \
             tc.tile_pool(name="xpool", bufs=2) as xpool, \
             tc.tile_pool(name="x1pool", bufs=2) as x1pool, \
             tc.tile_pool(name="hpool", bufs=2) as hpool, \
             tc.tile_pool(name="fpool", bufs=2) as fpool, \
             tc.tile_pool(name="hps", bufs=2, space="PSUM") as hps, \
             tc.tile_pool(name="rps", bufs=1, space="PSUM") as rps, \
             tc.tile_pool(name="fps", bufs=1, space="PSUM") as fps, \
             tc.tile_pool(name="aps", bufs=1, space="PSUM") as aps:

            # ---- constants: two packed DMAs ----
            wpk_sb = const.tile([P, NW * 2, H], f8)
            nc.sync.dma_start(
                out=wpk_sb[:],
                in_=wpk[:].rearrange("p (s n) -> p s n", s=NW * 2))
            bpk_sb = const.tile([P, NW], f32)
            nc.sync.dma_start(out=bpk_sb[:], in_=bpk[:])
            wres_pr = wpk_sb[:, 0:2, :]
            wfc_pr = wpk_sb[:, (NW - 1) * 2:NW * 2, :]
            bres_sb = bpk_sb[:, 0:1]
            bfc_sb = bpk_sb[:, NW - 1:NW]
            macc = const.tile([P, GPC * 4], f32)

            def bcast(ap2d, ncols):
                return ap2d.unsqueeze(1).broadcast_to((P, 2, ncols))

            def dma_at(g):
                """A^T DMA on the SP queue.  Graph 0's is quartered so its
                first DoubleRow pairs can start before the full 4MB lands."""
                if g == 0:
                    ats = []
                    for qq in range(4):
                        t = apool.tile([P, 4, N], f8, tag=f"atq{qq}",
                                       name=f"at0_{qq}")
                        nc.sync.dma_start(
                            out=t[:],
                            in_=at[0:P, qq * 4 * N:(qq + 1) * 4 * N].rearrange(
                                "p (s n) -> p s n", s=4))
                        ats.append(t)
                else:
                    t = apool.tile([P, NCH, N], f8, tag="at", name=f"at{g}")
                    nc.sync.dma_start(
                        out=t[:],
                        in_=at[g * P:(g + 1) * P, :].rearrange(
                            "p (s n) -> p s n", s=NCH))
                    ats = [t]
                return ats

            def at_pair(ats, j, q):
                if len(ats) == 4:
                    t, jj = ats[j // 2], (j % 2) * 2
                else:
                    t, jj = ats[0], 2 * j
                return t[:, jj:jj + 2, q * 512:(q + 1) * 512]

            def emit_res_q(g, xT, x1T, q):
                """Residual strip q for graph g: fp8 DR matmul + DVE relu
                (ACT is reserved for the layer-output strips and fc)."""
                rp = rps.tile([P, 512], f32, tag="rps", name=f"rp{g}_{q}")
                nc.tensor.matmul(out=rp[:], lhsT=wres_pr,
                                 rhs=bcast(xT[:, q * 512:(q + 1) * 512], 512),
                                 start=True, stop=True, perf_mode=DR)
                nc.vector.tensor_scalar(
                    out=x1T[:, q * 512:(q + 1) * 512], in0=rp[:],
                    scalar1=bres_sb, scalar2=0.0,
                    op0=mybir.AluOpType.add, op1=mybir.AluOpType.max)

            def emit_hgrp_q(g, l, x_src, h8t, q):
                """h-group q of layer l (chunks 4q..4q+3) + fp8 bulk cast."""
                hp = hps.tile([P, 512], f32, tag="hps", name=f"hp{g}_{l}_{q}")
                for c in range(4):
                    j = q * 4 + c
                    nc.tensor.matmul(
                        out=hp[:, c * H:(c + 1) * H],
                        lhsT=bcast(x_src[:, j * P:(j + 1) * P], P),
                        rhs=wpk_sb[:, (1 + l) * 2:(2 + l) * 2, :],
                        start=(c == 0), stop=(c == 3), perf_mode=DR)
                nc.vector.tensor_copy(
                    out=h8t[:, q * 4:(q + 1) * 4, :].rearrange(
                        "p s f -> p (s f)"),
                    in_=hp[:])

            def emit_fc_q(g, xn, x1T, q):
                """fc1 strip q: two bf16 matmuls accumulating (layer output
                + residual), then ACT relu + node-sum into macc."""
                fp = fps.tile([P, 512], f32, tag="fps", name=f"fp{g}_{q}")
                nc.tensor.matmul(out=fp[:], lhsT=wfc_pr,
                                 rhs=bcast(xn[:, q * 512:(q + 1) * 512], 512),
                                 start=True, stop=False, perf_mode=DR)
                nc.tensor.matmul(out=fp[:], lhsT=wfc_pr,
                                 rhs=bcast(x1T[:, q * 512:(q + 1) * 512], 512),
                                 start=False, stop=True, perf_mode=DR)
                fcq = fpool.tile([P, 512], f32, tag="fcq", name=f"fc{g}_{q}")
                nc.scalar.activation(
                    out=fcq[:], in_=fp[:], func=RELU, bias=bfc_sb,
                    accum_out=macc[:, g * 4 + q:g * 4 + q + 1])

            # ---- graph 0 prologue: inputs + residual + layer-0 h ----
            xT0 = inpool.tile([P, N], f8, tag="xin", name="x0_0")
            nc.sync.dma_start(out=xT0[:], in_=x0[:, 0:N])
            ats = dma_at(0)
            xT = xT0[:]
            x1T = x1pool.tile([P, N], f8, tag="x1", name="x1_0")
            h8 = hpool.tile([P, NCH, H], f8, tag="h", name="h0_0")
            for q in range(4):
                emit_res_q(0, xT, x1T, q)
                emit_hgrp_q(0, 0, xT, h8, q)

            # Deferred emission: `pending` thunks (trailing h-group/prologue
            # pieces) flush at pair j=1 of the NEXT pair loop; fc strips of
            # graph g flush inside graph g+1's first pair loop, where PE has
            # independent DoubleRow work to hide the ACT drain.
            pending = []
            pending_fc = []
            xall_sb = None

            for g in range(GPC):
                nxt = g + 1 < GPC
                if nxt:
                    ats_n = dma_at(g + 1)
                    if g == 0 and GPC > 1:
                        # all remaining graphs' features in one DMA, queued
                        # right after at(1) on SP
                        xall_sb = const.tile([P, (GPC - 1) * N], f8)
                        nc.sync.dma_start(out=xall_sb[:],
                                          in_=x0[:, N:GPC * N])
                    xT_n = xall_sb[:, g * N:(g + 1) * N]
                    x1T_n = x1pool.tile([P, N], f8, tag="x1",
                                        name=f"x1_{g + 1}")
                    h8_n0 = hpool.tile([P, NCH, H], f8, tag="h",
                                       name=f"h{g + 1}_0")
                    # graph 0's layer 0 is DMA-paced, so it hosts the next
                    # graph's prologue; later graphs weave it into their
                    # last layer's strip-finish phase
                    wl = 0 if (g == 0 and n_layers > 1) else n_layers - 1

                    def pre_piece(q, _xT=xT_n, _x1=x1T_n, _h8=h8_n0, _g=g + 1):
                        emit_res_q(_g, _xT, _x1, q)
                        emit_hgrp_q(_g, 0, _xT, _h8, q)
                else:
                    wl = -1

                for l in range(n_layers):
                    last = (l == n_layers - 1)
                    ps_l = [aps.tile([P, 512], f32, tag=f"aps{q}",
                                     name=f"as{g}_{l}_{q}") for q in range(4)]
                    # pairs 0-5 interleaved across strips: tolerates the
                    # trailing h-casts of this layer still landing
                    for j in range(6):
                        hj = h8[:, 2 * j:2 * j + 2, :]
                        for q in range(4):
                            nc.tensor.matmul(
                                out=ps_l[q][:], lhsT=hj,
                                rhs=at_pair(ats, j, q),
                                start=(j == 0), stop=False, perf_mode=DR)
                        if j == 1:
                            for fn in pending:
                                fn()
                            pending = []
                        if l == 0 and 1 <= j <= 4 and pending_fc:
                            pending_fc[j - 1]()
                            if j == 4:
                                pending_fc = []
                        if l == wl and wl == 0 and n_layers > 1 and j in (1, 3, 5):
                            pre_piece(j // 2)
                    # finish strips one at a time; successor work for strip q
                    # is deferred one strip so it lands after ACT completes
                    xn = xpool.tile([P, N], f8, tag="x", name=f"x{g}_{l}")
                    if not last:
                        h8n = hpool.tile([P, NCH, H], f8, tag="h",
                                         name=f"h{g}_{l + 1}")

                    def deferred(q, _l=l, _last=last, _wl=wl, _xn=xn,
                                 _h8n=(None if last else h8n), _g=g,
                                 _pp=(pre_piece if nxt else None)):
                        if not _last:
                            emit_hgrp_q(_g, _l + 1, _xn, _h8n, q)
                        elif (_l == _wl and n_layers > 1) or (
                                _wl == 0 and n_layers == 1):
                            _pp(q)

                    extra3 = (l == wl and wl == 0 and n_layers > 1)
                    for q in range(4):
                        for j in (6, 7):
                            nc.tensor.matmul(
                                out=ps_l[q][:],
                                lhsT=h8[:, 2 * j:2 * j + 2, :],
                                rhs=at_pair(ats, j, q),
                                start=False, stop=(j == 7), perf_mode=DR)
                        nc.scalar.activation(
                            out=xn[:, q * 512:(q + 1) * 512], in_=ps_l[q][:],
                            func=RELU, bias=bpk_sb[:, 1 + l:2 + l])
                        if q >= 1:
                            deferred(q - 1)
                    pending.append(lambda _d=deferred: _d(3))
                    if extra3:
                        pending.append(lambda _pp=pre_piece: _pp(3))
                    if last:
                        pending_fc = [
                            (lambda _q=q, _xn=xn, _x1=x1T, _g=g:
                             emit_fc_q(_g, _xn, _x1, _q)) for q in range(4)]
                    else:
                        h8 = h8n
                if nxt:
                    xT, ats, x1T, h8 = xT_n, ats_n, x1T_n, h8_n0

            for fn in pending:
                fn()
            for fn in pending_fc:
                fn()
            nc.sync.dma_start(out=macc_out[:], in_=macc[:])

    nc.compile()
    return nc


class _Runner:
    """Compile once, keep the jitted sharded executable for repeat calls."""

    def __init__(self, n_layers: int):
        import jax
        from jax.sharding import Mesh, PartitionSpec
        from jax.experimental.shard_map import shard_map

        self.jax = jax
        nc = _build_program(n_layers)
        self.nc = nc
        bass2jax.install_neuronx_cc_hook()

        in_names, out_names, out_avals, zero_outs = [], [], [], []
        pid_name = nc.partition_id_tensor.name if nc.partition_id_tensor else None
        for alloc in nc.m.functions[0].allocations:
            if not isinstance(alloc, mybir.MemoryLocationSet):
                continue
            name = alloc.memorylocations[0].name
            if alloc.kind == "ExternalInput":
                if name != pid_name:
                    in_names.append(name)
            elif alloc.kind == "ExternalOutput":
                out_names.append(name)
                shape = tuple(alloc.tensor_shape)
                dtype = mybir.dt.np(alloc.dtype)
                out_avals.append(jax.core.ShapedArray(shape, dtype))
                zero_outs.append(np.zeros(shape, dtype))
        self.in_names = list(in_names)
        self.out_names = out_names
        self.zero_outs = zero_outs
        n_params = len(in_names)
        all_names = in_names + out_names + ([pid_name] if pid_name else [])

        def _body(*args):
            operands = list(args)
            if pid_name is not None:
                operands.append(bass2jax.partition_id_tensor())
            return tuple(bass2jax._bass_exec_p.bind(
                *operands,
                out_avals=tuple(out_avals),
                in_names=tuple(all_names),
                out_names=tuple(out_names),
                lowering_input_output_aliases=(),
                sim_require_finite=True,
                sim_require_nnan=True,
                nc=nc,
            ))

        devices = jax.devices()[:N_CORES]
        mesh = Mesh(np.asarray(devices), ("core",))
        self.fn = jax.jit(
            shard_map(_body, mesh=mesh,
                      in_specs=(PartitionSpec("core"),) * (n_params + len(out_names)),
                      out_specs=(PartitionSpec("core"),) * len(out_names),
                      check_rep=False),
            keep_unused=True)

    def run(self, concat_inputs: list[np.ndarray]):
        jax = self.jax
        concat_zeros = [np.zeros((N_CORES * z.shape[0], *z.shape[1:]), z.dtype)
                        for z in self.zero_outs]
        outs = self.fn(*concat_inputs, *concat_zeros)
        jax.block_until_ready(outs)
        return {name: np.asarray(outs[i]) for i, name in enumerate(self.out_names)}


_RUNNERS: dict[int, _Runner] = {}


def _prepare_inputs(all_features, feature_index, edge_index,
                    lin_res_w, lin_res_b, gcn_w, gcn_b,
                    fc1_w, fc1_b, lin_w, lin_b, n_layers):
    """Build the concatenated (over cores, axis 0) device input list."""
    feats = np.asarray(all_features, np.float32)
    fi = np.asarray(feature_index).astype(np.int64)
    ei = np.asarray(edge_index).astype(np.int32)

    # host gather + transpose to feature-major fp8 [G, 128, 2048]
    x0_all = np.ascontiguousarray(
        feats[fi].transpose(0, 2, 1)).astype(E4NP)          # [G, D, N]

    # A^T per graph: accumulate duplicate (src,dst) cells, quantize fp8,
    # swizzle to [128 part, 16 chunk, 2048 dst].
    at_all = np.zeros((G, N * N), np.float32)
    diag_keys = (np.arange(N, dtype=np.int64) * (N + 1)).astype(np.int32)
    for g in range(G):
        src = ei[g, 0]
        dst = ei[g, 1]
        deg = np.bincount(dst, minlength=N).astype(np.float32) + 1.0
        dinv = 1.0 / np.sqrt(deg)
        coef = dinv[src] * dinv[dst]
        keys = np.concatenate([src.astype(np.int32) * N + dst, diag_keys])
        vals = np.concatenate([coef, dinv * dinv]).astype(np.float64)
        order = np.argsort(keys, kind="stable")
        ks, vs = keys[order], vals[order]
        first = np.empty(len(ks), bool)
        first[0] = True
        first[1:] = ks[1:] != ks[:-1]
        starts = np.nonzero(first)[0]
        sums = np.add.reduceat(vs, starts).astype(np.float32)
        np.put(at_all[g], ks[starts], sums)
    at8 = at_all.reshape(G, NCH, P, N).transpose(0, 2, 1, 3)  # [G,128,16,2048]
    at8 = np.ascontiguousarray(at8).astype(E4NP).reshape(G, P, NCH * N)

    # packed fp8 weight-compensation pairs [128, (2+L)*2*128]:
    # [W8 | (W-W8)8] blocks for wres | gw[0..L) | wfc
    NW = 2 + n_layers
    wpk = np.empty((P, NW * 2 * H), E4NP)

    def put_pair(b, W):
        Wf = np.asarray(W, np.float32)
        W8 = Wf.astype(E4NP)
        wpk[:, (2 * b) * H:(2 * b + 1) * H] = W8
        wpk[:, (2 * b + 1) * H:(2 * b + 2) * H] = (
            (Wf - W8.astype(np.float32)).astype(E4NP))

    put_pair(0, lin_res_w)
    for l in range(n_layers):
        put_pair(1 + l, gcn_w[l])
    put_pair(NW - 1, fc1_w)
    # packed biases [128, 2+L] f32: bres | gb[0..L) | bfc
    bpk = np.empty((P, NW), np.float32)
    bpk[:, 0] = np.asarray(lin_res_b, np.float32)
    for l in range(n_layers):
        bpk[:, 1 + l] = np.asarray(gcn_b[l], np.float32)
    bpk[:, NW - 1] = np.asarray(fc1_b, np.float32)

    per_core = {}
    per_core["x0"] = [np.ascontiguousarray(
        x0_all[c * GPC:(c + 1) * GPC].transpose(1, 0, 2)).reshape(P, GPC * N)
        for c in range(N_CORES)]
    per_core["at"] = [at8[c * GPC:(c + 1) * GPC].reshape(GPC * P, NCH * N)
                      for c in range(N_CORES)]
    per_core["wpk"] = [wpk] * N_CORES
    per_core["bpk"] = [bpk] * N_CORES
    return per_core


def kernel(all_features, feature_index, edge_index, action,
           lin_res_w, lin_res_b, gcn_w, gcn_b,
           fc1_w, fc1_b, lin_w, lin_b):
    n_layers = int(action) + 1
    assert 1 <= n_layers <= 3

    if n_layers not in _RUNNERS:
        _RUNNERS[n_layers] = _Runner(n_layers)
    runner = _RUNNERS[n_layers]

    per_core = _prepare_inputs(
        all_features, feature_index, edge_index,
        lin_res_w, lin_res_b, gcn_w, gcn_b, fc1_w, fc1_b, lin_w, lin_b,
        n_layers)

    concat = [np.concatenate(per_core[name], axis=0)
              for name in runner.in_names]
    outs = runner.run(concat)

    # host head: node-sums -> means -> logits -> log_softmax
    macc = outs["macc_out"].reshape(N_CORES, P, GPC, 4)
    means = macc.sum(axis=3).transpose(0, 2, 1).reshape(G, H) / N   # [G, H]
    lg = means @ np.asarray(lin_w, np.float32) + np.asarray(lin_b, np.float32)
    mx = lg.max(axis=1, keepdims=True)
    ls = lg - mx - np.log(np.exp(lg - mx).sum(axis=1, keepdims=True))
    return np.asarray(ls, np.float32), np.asarray(lg, np.float32)


# revision 32
# speedup vs baseline: 1.1138x; 1.1138x over previous
"""GCN (3-layer, PyG GCNConv-style) forward on 8 Trainium2 NeuronCores.

Strategy: data-parallel over the 64 graphs (8 graphs per core).  The
message-passing scatter-add is a dense normalized-adjacency matmul run in
fp8e4m3 with MatmulPerfMode.DoubleRow (two 128-row k-tiles per instruction,
0.5 cycles/row), which is 4x the fp32r FLOP rate for the dominant A@h
product.  Weight-side matmuls stay bf16 (weight quantization error is
systematic across nodes and does not average out; fp8 weights blow the
error budget, bf16 lands ~1.6e-3 on the logits).

Host-side prep: the feature gather from the 500k-row table and the dense
A^T build happen on the host; the device receives per-graph feature tiles
(bf16, feature-major [128, 2048]) and A^T tiles (fp8, [128 src-part,
16 src-chunk, 2048 dst] swizzle) so each graph needs exactly two large
contiguous DMAs (features on the Pool DGE queue, A^T on SP so their
fixed per-DMA delays overlap).  Per layer on device:
    h   = x @ W        (16 bf16 matmuls, 4-chunk PSUM groups, bulk-cast
                        to fp8 on DVE)
    x'  = relu(A @ h + b)  (32 fp8 DoubleRow matmuls into 4 psum strips,
                        relu+bias on ACT writing bf16)
The layer orientations alternate (feat-major <-> node-major) so no
transposes are needed anywhere.  The device emits only the per-graph
node-sum accumulators (ACT accum_out); the 64x2 logits head and the
log_softmax run on the host.
"""

import os
import sys

for _p in ("/opt/trn_rl_repo", "/root/.axon_site/_ro/trn_rl_repo"):
    if os.path.isdir(_p) and _p not in sys.path:
        sys.path.insert(0, _p)

import numpy as np
import ml_dtypes

import concourse.bass as bass
import concourse.bacc as bacc
import concourse.mybir as mybir
import concourse.tile as tile
from concourse import bass2jax

G, N, E = 64, 2048, 32768
D = H = 128
O = 2
ALL = 500_000
P = 128
N_CORES = 8
GPC = G // N_CORES          # graphs per core
NCH = N // P                # 128-row chunks per graph (16)

f32 = mybir.dt.float32
bf16 = mybir.dt.bfloat16
f8 = mybir.dt.float8e4

E4NP = ml_dtypes.float8_e4m3      # == mybir.dt.np(float8e4)
BFNP = ml_dtypes.bfloat16

DR = mybir.MatmulPerfMode.DoubleRow
RELU = mybir.ActivationFunctionType.Relu


def _build_program(n_layers: int):
    nc = bacc.Bacc("TRN2", target_bir_lowering=False, debug=False,
                   num_devices=N_CORES)

    # packed fp8 weight-compensation pairs [W8 | (W - W8)8]: wres, gw[l]s,
    # wfc.  Every W-side matmul runs fp8 DoubleRow with the input broadcast
    # (stride-0) across the two k-tiles, recovering ~bf16 weight precision
    # at 2x the bf16 matmul rate.
    NW = 2 + n_layers
    x0 = nc.dram_tensor("x0", [P, GPC * N], f8, kind="ExternalInput")
    at = nc.dram_tensor("at", [GPC * P, NCH * N], f8, kind="ExternalInput")
    wpk = nc.dram_tensor("wpk", [P, NW * 2 * H], f8, kind="ExternalInput")
    bpk = nc.dram_tensor("bpk", [P, NW], f32, kind="ExternalInput")
    macc_out = nc.dram_tensor("macc_out", [P, GPC * 4], f32,
                              kind="ExternalOutput")

    with tile.TileContext(nc) as tc:
        with tc.tile_pool(name="const", bufs=1) as const, \
             tc.tile_pool(name="apool", bufs=2) as apool, \
             tc.tile_pool(name="inpool", bufs=2) as inpool, \
             tc.tile_pool(name="xpool", bufs=2) as xpool, \
             tc.tile_pool(name="x1pool", bufs=2) as x1pool, \
             tc.tile_pool(name="hpool", bufs=2) as hpool, \
             tc.tile_pool(name="fpool", bufs=2) as fpool, \
             tc.tile_pool(name="hps", bufs=2, space="PSUM") as hps, \
             tc.tile_pool(name="rps", bufs=1, space="PSUM") as rps, \
             tc.tile_pool(name="fps", bufs=1, space="PSUM") as fps, \
             tc.tile_pool(name="aps", bufs=1, space="PSUM") as aps:

            # ---- constants: two packed DMAs ----
            wpk_sb = const.tile([P, NW * 2, H], f8)
            nc.sync.dma_start(
                out=wpk_sb[:],
                in_=wpk[:].rearrange("p (s n) -> p s n", s=NW * 2))
            bpk_sb = const.tile([P, NW], f32)
            nc.sync.dma_start(out=bpk_sb[:], in_=bpk[:])
            wres_pr = wpk_sb[:, 0:2, :]
            wfc_pr = wpk_sb[:, (NW - 1) * 2:NW * 2, :]
            bres_sb = bpk_sb[:, 0:1]
            bfc_sb = bpk_sb[:, NW - 1:NW]
            macc = const.tile([P, GPC * 4], f32)

            def bcast(ap2d, ncols):
                return ap2d.unsqueeze(1).broadcast_to((P, 2, ncols))

            def dma_at(g):
                """A^T DMA on the SP queue.  Graph 0's is quartered so its
                first DoubleRow pairs can start before the full 4MB lands."""
                if g == 0:
                    ats = []
                    for qq in range(4):
                        t = apool.tile([P, 4, N], f8, tag=f"atq{qq}",
                                       name=f"at0_{qq}")
                        nc.sync.dma_start(
                            out=t[:],
                            in_=at[0:P, qq * 4 * N:(qq + 1) * 4 * N].rearrange(
                                "p (s n) -> p s n", s=4))
                        ats.append(t)
                else:
                    t = apool.tile([P, NCH, N], f8, tag="at", name=f"at{g}")
                    nc.sync.dma_start(
                        out=t[:],
                        in_=at[g * P:(g + 1) * P, :].rearrange(
                            "p (s n) -> p s n", s=NCH))
                    ats = [t]
                return ats

            def at_pair(ats, j, q):
                if len(ats) == 4:
                    t, jj = ats[j // 2], (j % 2) * 2
                else:
                    t, jj = ats[0], 2 * j
                return t[:, jj:jj + 2, q * 512:(q + 1) * 512]

            def emit_res_q(g, xT, x1T, q):
                """Residual strip q for graph g: fp8 DR matmul + DVE relu
                (ACT is reserved for the layer-output strips and fc)."""
                rp = rps.tile([P, 512], f32, tag="rps", name=f"rp{g}_{q}")
                nc.tensor.matmul(out=rp[:], lhsT=wres_pr,
                                 rhs=bcast(xT[:, q * 512:(q + 1) * 512], 512),
                                 start=True, stop=True, perf_mode=DR)
                nc.vector.tensor_scalar(
                    out=x1T[:, q * 512:(q + 1) * 512], in0=rp[:],
                    scalar1=bres_sb, scalar2=0.0,
                    op0=mybir.AluOpType.add, op1=mybir.AluOpType.max)

            def emit_hgrp_q(g, l, x_src, h8t, q):
                """h-group q of layer l (chunks 4q..4q+3) + fp8 bulk cast."""
                hp = hps.tile([P, 512], f32, tag="hps", name=f"hp{g}_{l}_{q}")
                for c in range(4):
                    j = q * 4 + c
                    nc.tensor.matmul(
                        out=hp[:, c * H:(c + 1) * H],
                        lhsT=bcast(x_src[:, j * P:(j + 1) * P], P),
                        rhs=wpk_sb[:, (1 + l) * 2:(2 + l) * 2, :],
                        start=(c == 0), stop=(c == 3), perf_mode=DR)
                nc.vector.tensor_copy(
                    out=h8t[:, q * 4:(q + 1) * 4, :].rearrange(
                        "p s f -> p (s f)"),
                    in_=hp[:])

            def emit_fc_q(g, xn, x1T, q):
                """fc1 strip q: two bf16 matmuls accumulating (layer output
                + residual), then ACT relu + node-sum into macc."""
                fp = fps.tile([P, 512], f32, tag="fps", name=f"fp{g}_{q}")
                nc.tensor.matmul(out=fp[:], lhsT=wfc_pr,
                                 rhs=bcast(xn[:, q * 512:(q + 1) * 512], 512),
                                 start=True, stop=False, perf_mode=DR)
                nc.tensor.matmul(out=fp[:], lhsT=wfc_pr,
                                 rhs=bcast(x1T[:, q * 512:(q + 1) * 512], 512),
                                 start=False, stop=True, perf_mode=DR)
                fcq = fpool.tile([P, 512], f32, tag="fcq", name=f"fc{g}_{q}")
                nc.scalar.activation(
                    out=fcq[:], in_=fp[:], func=RELU, bias=bfc_sb,
                    accum_out=macc[:, g * 4 + q:g * 4 + q + 1])

            def dma_x(g):
                t = inpool.tile([P, N], f8, tag="xin", name=f"x0_{g}")
                nc.sync.dma_start(out=t[:], in_=x0[:, g * N:(g + 1) * N])
                return t

            # ---- graph 0 prologue: inputs + residual + layer-0 h ----
            xT = dma_x(0)
            ats = dma_at(0)
            x1T = x1pool.tile([P, N], f8, tag="x1", name="x1_0")
            h8 = hpool.tile([P, NCH, H], f8, tag="h", name="h0_0")
            for q in range(4):
                emit_res_q(0, xT, x1T, q)
                emit_hgrp_q(0, 0, xT, h8, q)

            # Deferred emission: `pending` thunks (trailing h-group/prologue
            # pieces) flush at pair j=1 of the NEXT pair loop; fc strips of
            # graph g flush inside graph g+1's first pair loop, where PE has
            # independent DoubleRow work to hide the ACT drain.
            pending = []

            for g in range(GPC):
                nxt = g + 1 < GPC
                if nxt:
                    xT_n = dma_x(g + 1)
                    ats_n = dma_at(g + 1)
                    x1T_n = x1pool.tile([P, N], f8, tag="x1",
                                        name=f"x1_{g + 1}")
                    h8_n0 = hpool.tile([P, NCH, H], f8, tag="h",
                                       name=f"h{g + 1}_0")
                    # graph 0's layer 0 is DMA-paced, so it hosts the next
                    # graph's prologue; later graphs weave it into their
                    # last layer's strip-finish phase
                    wl = 0 if (g == 0 and n_layers > 1) else n_layers - 1

                    def pre_piece(q, _xT=xT_n, _x1=x1T_n, _h8=h8_n0, _g=g + 1):
                        emit_res_q(_g, _xT, _x1, q)
                        emit_hgrp_q(_g, 0, _xT, _h8, q)
                else:
                    wl = -1

                for l in range(n_layers):
                    last = (l == n_layers - 1)
                    ps_l = [aps.tile([P, 512], f32, tag=f"aps{q}",
                                     name=f"as{g}_{l}_{q}") for q in range(4)]
                    # pairs 0-5 interleaved across strips: tolerates the
                    # trailing h-casts of this layer still landing
                    for j in range(6):
                        hj = h8[:, 2 * j:2 * j + 2, :]
                        for q in range(4):
                            nc.tensor.matmul(
                                out=ps_l[q][:], lhsT=hj,
                                rhs=at_pair(ats, j, q),
                                start=(j == 0), stop=False, perf_mode=DR)
                        if j == 1:
                            for fn in pending:
                                fn()
                            pending = []
                        if l == wl and wl == 0 and n_layers > 1 and j in (1, 3, 5):
                            pre_piece(j // 2)
                    # finish strips one at a time; successor work for strip q
                    # is deferred one strip so it lands after ACT completes
                    xn = xpool.tile([P, N], f8, tag="x", name=f"x{g}_{l}")
                    if not last:
                        h8n = hpool.tile([P, NCH, H], f8, tag="h",
                                         name=f"h{g}_{l + 1}")

                    def deferred(q, _l=l, _last=last, _wl=wl, _xn=xn,
                                 _h8n=(None if last else h8n), _g=g, _x1=x1T,
                                 _pp=(pre_piece if nxt else None)):
                        if not _last:
                            emit_hgrp_q(_g, _l + 1, _xn, _h8n, q)
                        else:
                            if (_l == _wl and n_layers > 1) or (
                                    _wl == 0 and n_layers == 1):
                                _pp(q)
                            emit_fc_q(_g, _xn, _x1, q)

                    extra3 = (l == wl and wl == 0 and n_layers > 1)
                    for q in range(4):
                        for j in (6, 7):
                            nc.tensor.matmul(
                                out=ps_l[q][:],
                                lhsT=h8[:, 2 * j:2 * j + 2, :],
                                rhs=at_pair(ats, j, q),
                                start=False, stop=(j == 7), perf_mode=DR)
                        nc.scalar.activation(
                            out=xn[:, q * 512:(q + 1) * 512], in_=ps_l[q][:],
                            func=RELU, bias=bpk_sb[:, 1 + l:2 + l])
                        if q >= 1:
                            deferred(q - 1)
                    pending.append(lambda _d=deferred: _d(3))
                    if extra3:
                        pending.append(lambda _pp=pre_piece: _pp(3))
                    if not last:
                        h8 = h8n
                if nxt:
                    xT, ats, x1T, h8 = xT_n, ats_n, x1T_n, h8_n0

            for fn in pending:
                fn()
            nc.sync.dma_start(out=macc_out[:], in_=macc[:])

    nc.compile()
    return nc


class _Runner:
    """Compile once, keep the jitted sharded executable for repeat calls."""

    def __init__(self, n_layers: int):
        import jax
        from jax.sharding import Mesh, PartitionSpec
        from jax.experimental.shard_map import shard_map

        self.jax = jax
        nc = _build_program(n_layers)
        self.nc = nc
        bass2jax.install_neuronx_cc_hook()

        in_names, out_names, out_avals, zero_outs = [], [], [], []
        pid_name = nc.partition_id_tensor.name if nc.partition_id_tensor else None
        for alloc in nc.m.functions[0].allocations:
            if not isinstance(alloc, mybir.MemoryLocationSet):
                continue
            name = alloc.memorylocations[0].name
            if alloc.kind == "ExternalInput":
                if name != pid_name:
                    in_names.append(name)
            elif alloc.kind == "ExternalOutput":
                out_names.append(name)
                shape = tuple(alloc.tensor_shape)
                dtype = mybir.dt.np(alloc.dtype)
                out_avals.append(jax.core.ShapedArray(shape, dtype))
                zero_outs.append(np.zeros(shape, dtype))
        self.in_names = list(in_names)
        self.out_names = out_names
        self.zero_outs = zero_outs
        n_params = len(in_names)
        all_names = in_names + out_names + ([pid_name] if pid_name else [])

        def _body(*args):
            operands = list(args)
            if pid_name is not None:
                operands.append(bass2jax.partition_id_tensor())
            return tuple(bass2jax._bass_exec_p.bind(
                *operands,
                out_avals=tuple(out_avals),
                in_names=tuple(all_names),
                out_names=tuple(out_names),
                lowering_input_output_aliases=(),
                sim_require_finite=True,
                sim_require_nnan=True,
                nc=nc,
            ))

        devices = jax.devices()[:N_CORES]
        mesh = Mesh(np.asarray(devices), ("core",))
        self.fn = jax.jit(
            shard_map(_body, mesh=mesh,
                      in_specs=(PartitionSpec("core"),) * (n_params + len(out_names)),
                      out_specs=(PartitionSpec("core"),) * len(out_names),
                      check_rep=False),
            keep_unused=True)

    def run(self, concat_inputs: list[np.ndarray]):
        jax = self.jax
        concat_zeros = [np.zeros((N_CORES * z.shape[0], *z.shape[1:]), z.dtype)
                        for z in self.zero_outs]
        outs = self.fn(*concat_inputs, *concat_zeros)
        jax.block_until_ready(outs)
        return {name: np.asarray(outs[i]) for i, name in enumerate(self.out_names)}


_RUNNERS: dict[int, _Runner] = {}


def _prepare_inputs(all_features, feature_index, edge_index,
                    lin_res_w, lin_res_b, gcn_w, gcn_b,
                    fc1_w, fc1_b, lin_w, lin_b, n_layers):
    """Build the concatenated (over cores, axis 0) device input list."""
    feats = np.asarray(all_features, np.float32)
    fi = np.asarray(feature_index).astype(np.int64)
    ei = np.asarray(edge_index).astype(np.int32)

    # host gather + transpose to feature-major fp8 [G, 128, 2048]
    x0_all = np.ascontiguousarray(
        feats[fi].transpose(0, 2, 1)).astype(E4NP)          # [G, D, N]

    # A^T per graph: accumulate duplicate (src,dst) cells, quantize fp8,
    # swizzle to [128 part, 16 chunk, 2048 dst].
    at_all = np.zeros((G, N * N), np.float32)
    diag_keys = (np.arange(N, dtype=np.int64) * (N + 1)).astype(np.int32)
    for g in range(G):
        src = ei[g, 0]
        dst = ei[g, 1]
        deg = np.bincount(dst, minlength=N).astype(np.float32) + 1.0
        dinv = 1.0 / np.sqrt(deg)
        coef = dinv[src] * dinv[dst]
        keys = np.concatenate([src.astype(np.int32) * N + dst, diag_keys])
        vals = np.concatenate([coef, dinv * dinv]).astype(np.float64)
        order = np.argsort(keys, kind="stable")
        ks, vs = keys[order], vals[order]
        first = np.empty(len(ks), bool)
        first[0] = True
        first[1:] = ks[1:] != ks[:-1]
        starts = np.nonzero(first)[0]
        sums = np.add.reduceat(vs, starts).astype(np.float32)
        np.put(at_all[g], ks[starts], sums)
    at8 = at_all.reshape(G, NCH, P, N).transpose(0, 2, 1, 3)  # [G,128,16,2048]
    at8 = np.ascontiguousarray(at8).astype(E4NP).reshape(G, P, NCH * N)

    # packed fp8 weight-compensation pairs [128, (2+L)*2*128]:
    # [W8 | (W-W8)8] blocks for wres | gw[0..L) | wfc
    NW = 2 + n_layers
    wpk = np.empty((P, NW * 2 * H), E4NP)

    def put_pair(b, W):
        Wf = np.asarray(W, np.float32)
        W8 = Wf.astype(E4NP)
        wpk[:, (2 * b) * H:(2 * b + 1) * H] = W8
        wpk[:, (2 * b + 1) * H:(2 * b + 2) * H] = (
            (Wf - W8.astype(np.float32)).astype(E4NP))

    put_pair(0, lin_res_w)
    for l in range(n_layers):
        put_pair(1 + l, gcn_w[l])
    put_pair(NW - 1, fc1_w)
    # packed biases [128, 2+L] f32: bres | gb[0..L) | bfc
    bpk = np.empty((P, NW), np.float32)
    bpk[:, 0] = np.asarray(lin_res_b, np.float32)
    for l in range(n_layers):
        bpk[:, 1 + l] = np.asarray(gcn_b[l], np.float32)
    bpk[:, NW - 1] = np.asarray(fc1_b, np.float32)

    per_core = {}
    per_core["x0"] = [np.ascontiguousarray(
        x0_all[c * GPC:(c + 1) * GPC].transpose(1, 0, 2)).reshape(P, GPC * N)
        for c in range(N_CORES)]
    per_core["at"] = [at8[c * GPC:(c + 1) * GPC].reshape(GPC * P, NCH * N)
                      for c in range(N_CORES)]
    per_core["wpk"] = [wpk] * N_CORES
    per_core["bpk"] = [bpk] * N_CORES
    return per_core


def kernel(all_features, feature_index, edge_index, action,
           lin_res_w, lin_res_b, gcn_w, gcn_b,
           fc1_w, fc1_b, lin_w, lin_b):
    n_layers = int(action) + 1
    assert 1 <= n_layers <= 3

    if n_layers not in _RUNNERS:
        _RUNNERS[n_layers] = _Runner(n_layers)
    runner = _RUNNERS[n_layers]

    per_core = _prepare_inputs(
        all_features, feature_index, edge_index,
        lin_res_w, lin_res_b, gcn_w, gcn_b, fc1_w, fc1_b, lin_w, lin_b,
        n_layers)

    concat = [np.concatenate(per_core[name], axis=0)
              for name in runner.in_names]
    outs = runner.run(concat)

    # host head: node-sums -> means -> logits -> log_softmax
    macc = outs["macc_out"].reshape(N_CORES, P, GPC, 4)
    means = macc.sum(axis=3).transpose(0, 2, 1).reshape(G, H) / N   # [G, H]
    lg = means @ np.asarray(lin_w, np.float32) + np.asarray(lin_b, np.float32)
    mx = lg.max(axis=1, keepdims=True)
    ls = lg - mx - np.log(np.exp(lg - mx).sum(axis=1, keepdims=True))
    return np.asarray(ls, np.float32), np.asarray(lg, np.float32)
